# revision 9
# baseline (speedup 1.0000x reference)
"""Transformer block (pre-norm attention + MLP) on 8 TRN2 NeuronCores — fp8 DR
with subnormal-residual error compensation.

Sharding: 8 cores = 4 batch x 2 seq halves (data parallel, no collectives).
Each core computes 1024 own query tokens; K/V built for all 2048 tokens of its
batch element (own tokens first, permutation-invariant softmax).

All big GEMMs run fp8e4m3 with DoubleRow perf mode (contraction pairs of 128
partitions per instruction). Weights are scaled by powers of two host-side so
fp8 operands sit near unit scale; inverse scales fold into activation scale
params / epilogue multiplies (exact exponent shifts). Softmax: exp(s/8-4ln2)
keeps ex in [0,16) for fp8; denominators come from a ones-column in V; the
/16 cancels in normalization.

The MLP carries error compensation: each quantized operand Q(a) is paired
with a subnormal-fp8 residual (a - Q(a)), and the matmuls accumulate
main + residual terms (z2, w1, w2, and the gelu output h). This cuts the
final error ~4.5x vs plain fp8 while staying on the 2x DoubleRow path.

Schedule: query-chunk-major attention; first-half proj/LN2/fc1/fc2 runs as
filler work inside the second attention half (Act exp stream = critical
path). QKV/V fills are JIT inside the first group; LN1 is split around the
first fills so attention starts early.
"""

from collections import deque
from contextlib import ExitStack

import numpy as np

try:
    import jax
    jax.config.update("jax_compilation_cache_dir", "/tmp/jax_bass_cache")
    jax.config.update("jax_persistent_cache_min_compile_time_secs", 0.0)
    jax.config.update("jax_persistent_cache_min_entry_size_bytes", -1)
except Exception:
    import jax

import concourse.bacc as bacc
import concourse.bass as bass
import concourse.mybir as mybir
import concourse.tile as tile
from concourse.masks import make_identity

FP32 = mybir.dt.float32
BF16 = mybir.dt.bfloat16
FP8 = mybir.dt.float8e4
AF = mybir.ActivationFunctionType
ALU = mybir.AluOpType
DR = mybir.MatmulPerfMode.DoubleRow

D = 1024
DT = 8
H = 16
HD = 64
HID = 4096
T_ALL = 2048
T_OWN = 1024
EPS = 1e-6
N_CORES = 8
NEG4LN2 = -2.772588722239781


def build_nc():
    nc = bacc.Bacc("TRN2", target_bir_lowering=False, debug=False,
                   num_devices=N_CORES)

    x = nc.dram_tensor("x", [T_ALL, D], BF16, kind="ExternalInput")
    # [128, 24 fb, 8 dt, 128] : fb 0-7 Q (perm), 8-15 K (perm), 16-23 V
    wqkv = nc.dram_tensor("wqkv", [128, 24, DT, 128], FP8, kind="ExternalInput")
    bq = nc.dram_tensor("bq", [128, 16], FP32, kind="ExternalInput")
    vbias = nc.dram_tensor("vbias", [D], BF16, kind="ExternalInput")
    wproj = nc.dram_tensor("wproj", [128, DT, D], FP8, kind="ExternalInput")
    bproj = nc.dram_tensor("bproj", [D], BF16, kind="ExternalInput")
    w1 = nc.dram_tensor("w1", [128, 32, DT, 128], FP8, kind="ExternalInput")
    w1lo = nc.dram_tensor("w1lo", [128, 32, DT, 128], FP8, kind="ExternalInput")
    b1 = nc.dram_tensor("b1", [HID], FP32, kind="ExternalInput")
    w2 = nc.dram_tensor("w2", [128, 32, D], FP8, kind="ExternalInput")
    w2lo = nc.dram_tensor("w2lo", [128, 32, D], FP8, kind="ExternalInput")
    b2 = nc.dram_tensor("b2", [D], BF16, kind="ExternalInput")
    y = nc.dram_tensor("y", [T_OWN, D], FP32, kind="ExternalOutput")

    with tile.TileContext(nc) as tc, ExitStack() as ctx:
        P = ctx.enter_context

        singles = P(tc.tile_pool(name="singles", bufs=1))
        statpool = P(tc.tile_pool(name="stat", bufs=3))
        znpool = P(tc.tile_pool(name="zn", bufs=2))
        xo_pool = P(tc.tile_pool(name="xop", bufs=3))
        es_keep = ExitStack()
        keepp = es_keep.enter_context(
            tc.tile_pool(name="keep", bufs=1, side="right"))

        # ---- constants ----
        ident = singles.tile([128, 128], BF16)
        make_identity(nc, ident)
        ones64 = singles.tile([1, 64], BF16)
        nc.vector.memset(ones64, 1.0)
        ones_bf = singles.tile([1, 128], BF16)
        nc.vector.memset(ones_bf, 1.0)
        eps_sb = singles.tile([128, 1], FP32)
        nc.vector.memset(eps_sb, EPS)
        ebias = singles.tile([128, 1], FP32)
        nc.vector.memset(ebias, NEG4LN2)
        bq_sb = singles.tile([128, 16], FP32)
        nc.sync.dma_start(out=bq_sb, in_=bq.ap())
        b1_sb = singles.tile([128, 32], FP32)
        nc.sync.dma_start(out=b1_sb, in_=b1.ap().rearrange("(f p) -> p f", p=128))
        bproj_sb = singles.tile([1, D], BF16)
        nc.sync.dma_start(out=bproj_sb, in_=bproj.ap().rearrange("(o f) -> o f", o=1))
        b2_sb = singles.tile([1, D], BF16)
        nc.sync.dma_start(out=b2_sb, in_=b2.ap().rearrange("(o f) -> o f", o=1))
        vbias_sb = singles.tile([128, D], BF16)
        nc.sync.dma_start(
            out=vbias_sb,
            in_=bass.AP(tensor=vbias, offset=0, ap=[[0, 128], [1, D]]),
        )

        # ---- long-lived tiles ----
        projw_sb = keepp.tile([128, DT, D], FP8, tag="projw")
        w2_sb = keepp.tile([128, 32, D], FP8, tag="w2sb")
        x2_all = keepp.tile([128, DT, D], BF16, tag="x2")
        z2T = keepp.tile([128, DT, T_OWN], FP8, tag="z2T")
        z2loT = keepp.tile([128, DT, T_OWN], FP8, tag="z2loT")

        es_ao = ExitStack()
        aop = es_ao.enter_context(tc.tile_pool(name="aop", bufs=1, side="right"))
        aoT = aop.tile([128, DT, T_OWN], FP8, tag="aoT")

        x_t = x.ap().rearrange("(t p) f -> p t f", p=128)

        es_kqv = ExitStack()
        kqvp = es_kqv.enter_context(tc.tile_pool(name="kqvp", bufs=1))
        KT = kqvp.tile([128, 4, 2, T_ALL], FP8, tag="kt")     # hgrp, half, tok
        QT = kqvp.tile([128, 4, 2, T_OWN], FP8, tag="qt")
        VP = kqvp.tile([128, 16, 16, 80], FP8, tag="vp")      # kt, head, hd|1
        vp_ones = VP[:, :, :, 64:65]
        nc.vector.memset(vp_ones, 1.0)

        wq_state, wv_state, h_tiles, e_state, fc2_ps = {}, {}, {}, {}, {}

        def ln_stats(xt):
            stats = statpool.tile([128, 2, 6], FP32, tag="stats")
            nc.vector.bn_stats(out=stats[:, 0, :], in_=xt[:, 0:512])
            nc.vector.bn_stats(out=stats[:, 1, :], in_=xt[:, 512:1024])
            mv = statpool.tile([128, 2], FP32, tag="mv")
            nc.vector.bn_aggr(out=mv, in_=stats)
            sd = statpool.tile([128, 1], FP32, tag="sd")
            nc.scalar.activation(out=sd, in_=mv[:, 1:2], func=AF.Sqrt,
                                 bias=eps_sb)
            rinv = statpool.tile([128, 1], FP32, tag="rinv")
            nc.vector.reciprocal(out=rinv, in_=sd)
            return mv, rinv

        def ln1_tile(trp, tt, z1T, on_act):
            """LN1 one 128-token tile of x (streamed) into z1T fp8."""
            xt = xo_pool.tile([128, D], BF16, tag="xo")
            nc.sync.dma_start(out=xt, in_=x_t[:, tt, :])
            mv, rinv = ln_stats(xt)
            zn = znpool.tile([128, D], BF16, tag="zn")
            nc.gpsimd.tensor_scalar(
                out=zn, in0=xt, scalar1=mv[:, 0:1], scalar2=rinv,
                op0=ALU.subtract, op1=ALU.mult,
            )
            for g in range(2):
                ps = trp.tile([128, 4, 128], BF16, tag="sps")
                for i in range(4):
                    nc.tensor.transpose(
                        ps[:, i, :],
                        zn[:, (4 * g + i) * 128:(4 * g + i + 1) * 128], ident)
                dst = z1T[:, 4 * g:4 * g + 4, tt * 128:(tt + 1) * 128]
                if on_act:
                    nc.scalar.copy(out=dst, in_=ps)
                else:
                    nc.vector.tensor_copy(out=dst, in_=ps)

        def kq_fill(wq_pool, psp, z1T, fb, hg, hf, dst, tcn):
            """One 512-token chunk of one K/Q feature block."""
            if fb not in wq_state:
                wq_f = wq_pool.tile([128, DT, 128], FP8, tag="wqf")
                nc.sync.dma_start(out=wq_f, in_=wqkv.ap()[:, fb, :, :])
                wq_state[fb] = wq_f
            wq_f = wq_state[fb]
            ps = psp.tile([128, 512], FP32, tag="qkps")
            for t in range(4):
                nc.tensor.matmul(
                    ps, wq_f[:, 2 * t:2 * t + 2, :],
                    z1T[:, 2 * t:2 * t + 2, tcn * 512:(tcn + 1) * 512],
                    start=(t == 0), stop=(t == 3), perf_mode=DR,
                )
            nc.vector.tensor_scalar(
                out=dst[:, hg, hf, tcn * 512:(tcn + 1) * 512],
                in0=ps, scalar1=bq_sb[:, fb:fb + 1],
                scalar2=None, op0=ALU.add,
            )

        def v_fill(wv_pool, psp, z1T, vc, tt):
            if vc not in wv_state:
                wv = wv_pool.tile([128, DT, 4, 128], FP8, tag="wvf")
                for b in range(4):
                    nc.sync.dma_start(
                        out=wv[:, :, b, :],
                        in_=wqkv.ap()[:, 16 + 4 * vc + b, :, :])
                wv_state[vc] = wv
            wv = wv_state[vc]
            ps = psp.tile([128, 512], FP32, tag="qkps")
            for t in range(4):
                nc.tensor.matmul(
                    ps, z1T[:, 2 * t:2 * t + 2, tt * 128:(tt + 1) * 128],
                    wv[:, 2 * t:2 * t + 2, :, :].rearrange("p d b f -> p d (b f)"),
                    start=(t == 0), stop=(t == 3), perf_mode=DR,
                )
            dst = VP[:, tt, vc * 8:(vc + 1) * 8, 0:64]
            srcp = ps.rearrange("p (h e) -> p h e", e=HD)
            vb = vbias_sb[:, vc * 512:(vc + 1) * 512].rearrange(
                "p (h e) -> p h e", e=HD)
            nc.vector.scalar_tensor_tensor(
                out=dst, in0=srcp, scalar=0.0, in1=vb,
                op0=ALU.bypass, op1=ALU.add,
            )

        def proj_fill(psp, tt, oc, ps_tag="qkps"):
            ps = psp.tile([128, 512], FP32, tag=ps_tag)
            for t in range(4):
                nc.tensor.matmul(
                    ps, aoT[:, 2 * t:2 * t + 2, tt * 128:(tt + 1) * 128],
                    projw_sb[:, 2 * t:2 * t + 2, oc * 512:(oc + 1) * 512],
                    start=(t == 0), stop=False, perf_mode=DR,
                )
            nc.tensor.matmul(
                ps, ones_bf, bproj_sb[:, oc * 512:(oc + 1) * 512],
                start=False, stop=True,
            )
            xt = xo_pool.tile([128, 512], BF16, tag="xo", name=f"xr{tt}_{oc}")
            nc.sync.dma_start(
                out=xt,
                in_=x_t[:, tt, oc * 512:(oc + 1) * 512])
            nc.vector.scalar_tensor_tensor(
                out=x2_all[:, tt, oc * 512:(oc + 1) * 512],
                in0=ps, scalar=2.0 ** -10, in1=xt,
                op0=ALU.mult, op1=ALU.add,
            )

        def e_stat1(base, i):
            if base not in e_state:
                mv4 = statpool.tile([128, 4, 2], FP32, tag="mv4",
                                    name=f"mv4_{base}")
                e_state[base] = [mv4, None]
            mv4 = e_state[base][0]
            stats = statpool.tile([128, 2, 6], FP32, tag="stats")
            xt = x2_all[:, base + i, :]
            nc.vector.bn_stats(out=stats[:, 0, :], in_=xt[:, 0:512])
            nc.vector.bn_stats(out=stats[:, 1, :], in_=xt[:, 512:1024])
            nc.vector.bn_aggr(out=mv4[:, i, :], in_=stats)

        def e_sqrt4(base):
            mv4 = e_state[base][0]
            sd4 = statpool.tile([128, 4], FP32, tag="sd4", name=f"sd4_{base}")
            nc.scalar.activation(out=sd4, in_=mv4[:, :, 1], func=AF.Sqrt,
                                 bias=eps_sb)
            rinv4 = statpool.tile([128, 4], FP32, tag="rinv4",
                                  name=f"rinv4_{base}")
            nc.vector.reciprocal(out=rinv4, in_=sd4)
            e_state[base][1] = rinv4

        def e_tr(psp, tt, ps_tag="qkps", on_act=False):
            """LN2 apply + transpose for one tile -> z2T fp8 + z2loT resid."""
            base = tt // 4 * 4
            mv4, rinv4 = e_state[base]
            i = tt - base
            zn = znpool.tile([128, D], BF16, tag="zn")
            nc.gpsimd.tensor_scalar(
                out=zn, in0=x2_all[:, tt, :], scalar1=mv4[:, i, 0:1],
                scalar2=rinv4[:, i:i + 1], op0=ALU.subtract, op1=ALU.mult,
            )
            for g in range(2):
                ps = psp.tile([128, 4, 128], BF16, tag=ps_tag)
                for i2 in range(4):
                    nc.tensor.transpose(
                        ps[:, i2, :],
                        zn[:, (4 * g + i2) * 128:(4 * g + i2 + 1) * 128], ident)
                d8 = z2T[:, 4 * g:4 * g + 4, tt * 128:(tt + 1) * 128]
                if on_act:
                    nc.scalar.copy(out=d8, in_=ps)
                else:
                    nc.vector.tensor_copy(out=d8, in_=ps)
                dlo = z2loT[:, 4 * g:4 * g + 4, tt * 128:(tt + 1) * 128]
                nc.vector.tensor_tensor(out=dlo, in0=ps, in1=d8,
                                        op=ALU.subtract)

        def fc1_fill(hpool, hb_pool, w1_pool, psp, tc2, jt, ps_tag="qkps",
                     wtag="wqf", pre=None):
            """fc1 with z2 + w1 compensation, gelu, then h8/hlo split on
            gpsimd (subnormal residual)."""
            if tc2 not in h_tiles:
                h8 = hpool.tile([128, 32, 512], FP8, tag="h8",
                                name=f"h8_{tc2}")
                hlo = hpool.tile([128, 32, 512], FP8, tag="hlo",
                                 name=f"hlo_{tc2}")
                h_tiles[tc2] = (h8, hlo)
            h8, hlo = h_tiles[tc2]
            if pre is not None:
                w1f, w1l = pre
            else:
                w1f = w1_pool.tile([128, DT, 128], FP8, tag=wtag,
                                   name=f"w1f_{tc2}_{jt}")
                nc.sync.dma_start(out=w1f, in_=w1.ap()[:, jt, :, :])
                w1l = w1_pool.tile([128, DT, 128], FP8, tag=wtag,
                                   name=f"w1l_{tc2}_{jt}")
                nc.sync.dma_start(out=w1l, in_=w1lo.ap()[:, jt, :, :])
            ps = psp.tile([128, 512], FP32, tag=ps_tag)
            sl = slice(tc2 * 512, (tc2 + 1) * 512)
            for t in range(4):
                nc.tensor.matmul(
                    ps, w1f[:, 2 * t:2 * t + 2, :],
                    z2T[:, 2 * t:2 * t + 2, sl],
                    start=(t == 0), stop=False, perf_mode=DR,
                )
            for t in range(4):
                nc.tensor.matmul(
                    ps, w1f[:, 2 * t:2 * t + 2, :],
                    z2loT[:, 2 * t:2 * t + 2, sl],
                    start=False, stop=False, perf_mode=DR,
                )
            for t in range(4):
                nc.tensor.matmul(
                    ps, w1l[:, 2 * t:2 * t + 2, :],
                    z2T[:, 2 * t:2 * t + 2, sl],
                    start=False, stop=(t == 3), perf_mode=DR,
                )
            hb = hb_pool.tile([128, 512], BF16, tag="hb")
            nc.scalar.activation(
                out=hb, in_=ps, func=AF.Gelu,
                scale=2.0 ** -5, bias=b1_sb[:, jt:jt + 1],
            )
            nc.vector.tensor_copy(out=h8[:, jt, :], in_=hb)
            nc.gpsimd.tensor_tensor(out=hlo[:, jt, :], in0=hb,
                                    in1=h8[:, jt, :], op=ALU.subtract)

        def fc2_sub(ypool, psp, w2lo_pool, tc2, tt, oc, half, sub,
                    ps_tag="qkps"):
            """Quarter of one fc2 chunk (4 pairs, 12 DR) with h and w2
            compensation; streams one 8-row w2-residual tile."""
            h8, hlo = h_tiles[tc2]
            tglob = tc2 * 4 + tt
            key = (tc2, tt, oc)
            if half == 0 and sub == 0:
                fc2_ps[key] = psp.tile([128, 512], FP32, tag=ps_tag,
                                       name=f"f2_{tc2}_{tt}_{oc}")
            ps = fc2_ps[key]
            tsl = slice(tt * 128, (tt + 1) * 128)
            osl = slice(oc * 512, (oc + 1) * 512)
            base = 16 * half + 8 * sub
            w2l = w2lo_pool.tile([128, 8, 512], FP8, tag="w2lo",
                                 name=f"w2l_{tc2}_{tt}_{oc}_{half}_{sub}")
            nc.sync.dma_start(out=w2l, in_=w2lo.ap()[:, base:base + 8, osl])
            for p in range(4):
                jt = 8 * half + 4 * sub + p
                jsl = slice(2 * jt, 2 * jt + 2)
                lsl = slice(2 * p, 2 * p + 2)
                nc.tensor.matmul(ps, h8[:, jsl, tsl], w2_sb[:, jsl, osl],
                                 start=(jt == 0), stop=False, perf_mode=DR)
                nc.tensor.matmul(ps, hlo[:, jsl, tsl], w2_sb[:, jsl, osl],
                                 start=False, stop=False, perf_mode=DR)
                nc.tensor.matmul(ps, h8[:, jsl, tsl], w2l[:, lsl, :],
                                 start=False, stop=False, perf_mode=DR)
            if not (half == 1 and sub == 1):
                return
            nc.tensor.matmul(
                ps, ones_bf, b2_sb[:, osl], start=False, stop=True,
            )
            ys = ypool.tile([128, 512], FP32, tag="ys")
            nc.vector.scalar_tensor_tensor(
                out=ys, in0=ps, scalar=2.0 ** -6,
                in1=x2_all[:, tglob, osl],
                op0=ALU.mult, op1=ALU.add,
            )
            nc.sync.dma_start(
                out=y[tglob * 128:(tglob + 1) * 128, osl], in_=ys,
            )

        # ---- attention + interleaved everything ----
        with (
            tc.tile_pool(name="wq", bufs=4, side="right") as wq_pool,
            tc.tile_pool(name="hbA", bufs=5) as hb_pool,
            tc.tile_pool(name="ypA", bufs=1) as ypoolA,
            tc.tile_pool(name="psB", bufs=1, space="PSUM") as qkpsum,
            tc.tile_pool(name="exs", bufs=1) as exp_pool,
            tc.tile_pool(name="nrm", bufs=2) as nrm_pool,
            tc.tile_pool(name="psS", bufs=2, space="PSUM") as spsum,
            tc.tile_pool(name="psAV", bufs=2, space="PSUM") as avpsum,
            tc.tile_pool(name="psBC", bufs=1, space="PSUM") as bcpsum,
        ):
            es_wv = ExitStack()
            wv_pool = es_wv.enter_context(
                tc.tile_pool(name="wv", bufs=2, side="right"))
            es_z1 = ExitStack()
            z1p = es_z1.enter_context(
                tc.tile_pool(name="z1p", bufs=1, side="right"))
            z1T = z1p.tile([128, DT, T_ALL], FP8, tag="z1T")

            def attn_group(hp, qc, fillers, per_kt=None, finish_prev=None):
                """Head pair (2hp, 2hp+1), query chunk qc. The previous
                group's last AV pair + normalization run deferred at kt==0
                so they don't gate this group's first scores/exp."""
                h0, h1 = 2 * hp, 2 * hp + 1
                hg = hp // 2
                EX4 = exp_pool.tile([128, 2, 4, 512], FP8, tag="ex4")
                avs = []
                for _i in range(2):
                    av_t = avpsum.tile([65, 512], FP32, tag="av")
                    avs.append(av_t)
                for kt in range(16):
                    sp = spsum.tile([128, 2, 512], FP32, tag="sps")
                    for i, h in enumerate((h0, h1)):
                        hr = h % 4
                        nc.tensor.matmul(
                            sp[:, i, :],
                            KT[32 * hr:32 * hr + 32, hg, :,
                               kt * 128:(kt + 1) * 128],
                            QT[32 * hr:32 * hr + 32, hg, :,
                               qc * 512:(qc + 1) * 512],
                            start=True, stop=True, perf_mode=DR,
                            tile_position=(32 * hr, 0),
                        )
                    nc.scalar.activation(
                        out=EX4[:, :, kt % 4, :], in_=sp, func=AF.Exp,
                        scale=2.0 ** -13, bias=ebias)
                    if kt == 0 and finish_prev is not None:
                        finish_prev()
                    if per_kt is not None:
                        per_kt(kt)
                    if kt % 2 == 1 and kt < 15:
                        s0 = 2 * ((kt // 2) % 2)
                        for i, h in enumerate((h0, h1)):
                            nc.tensor.matmul(
                                avs[i], VP[:, kt - 1:kt + 1, h, 0:65],
                                EX4[:, i, s0:s0 + 2, :],
                                start=(kt == 1), stop=False,
                                perf_mode=DR,
                            )
                    if fillers:
                        fillers.popleft()()

                def finish():
                    for i, h in enumerate((h0, h1)):
                        nc.tensor.matmul(
                            avs[i], VP[:, 14:16, h, 0:65],
                            EX4[:, i, 2:4, :],
                            start=False, stop=True, perf_mode=DR,
                        )
                    for i, h in enumerate((h0, h1)):
                        av = avs[i]
                        rec = nrm_pool.tile([1, 512], BF16, tag="rec")
                        with nc.allow_low_precision(reason="softmax denom"):
                            nc.vector.reciprocal(out=rec, in_=av[64:65, :])
                        bps = bcpsum.tile([64, 512], FP32, tag="bps")
                        nc.tensor.matmul(bps, ones64, rec, start=True,
                                         stop=True)
                        bcs = nrm_pool.tile([64, 512], BF16, tag="bcs")
                        nc.vector.tensor_copy(out=bcs, in_=bps)
                        pr, ft = (h % 2) * 64, h // 2
                        nc.vector.scalar_tensor_tensor(
                            out=aoT[pr:pr + 64, ft, qc * 512:(qc + 1) * 512],
                            in0=av[0:64, :], scalar=0.0, in1=bcs,
                            op0=ALU.bypass, op1=ALU.mult,
                        )
                return finish

            def kqf(hg, hf, tcn, q=False):
                fb = (2 * hg + hf) if q else (8 + 2 * hg + hf)
                dst = QT if q else KT
                return lambda: kq_fill(wq_pool, qkpsum, z1T, fb, hg, hf,
                                       dst, tcn)

            def vf(vc, tt):
                return lambda: v_fill(wv_pool, qkpsum, z1T, vc, tt)

            # phase A: LN1 tiles (drain on Act: it is idle here)
            for tt in range(16):
                ln1_tile(spsum, tt, z1T, on_act=True)
            kqf(0, 0, 0)(); kqf(0, 1, 0)()
            kqf(0, 0, 0, q=True)(); kqf(0, 1, 0, q=True)()
            vf(0, 0)()
            nc.sync.dma_start(out=projw_sb, in_=wproj.ap())
            nc.sync.dma_start(out=w2_sb, in_=w2.ap())

            def per_kt_first(kt):
                jit = {0: kqf(0, 0, 1), 1: kqf(0, 1, 1),
                       2: kqf(0, 0, 2), 3: kqf(0, 1, 2),
                       4: kqf(0, 0, 1, q=True), 5: kqf(0, 1, 1, q=True),
                       6: kqf(0, 0, 3), 7: kqf(0, 1, 3)}
                if kt in jit:
                    jit[kt]()
                if kt < 15:
                    vf(0, kt + 1)()

            qc0_fillers = {
                1: deque([kqf(1, hf, tcn) for hf in range(2) for tcn in range(4)]
                         + [kqf(1, hf, tcn, q=True) for hf in range(2)
                            for tcn in range(2)]),
                2: deque([vf(1, tt) for tt in range(8)]
                         + [kqf(2, hf, tcn) for hf in range(2) for tcn in (0, 1)]),
                3: deque([vf(1, tt) for tt in range(8, 16)]
                         + [kqf(2, hf, tcn) for hf in range(2) for tcn in (2, 3)]
                         + [kqf(2, hf, tcn, q=True) for hf in range(2)
                            for tcn in range(2)]),
                4: deque([kqf(3, hf, tcn) for hf in range(2) for tcn in range(4)]
                         + [kqf(3, hf, tcn, q=True) for hf in range(2)
                            for tcn in range(2)]),
            }
            fin = None
            for hp in range(8):
                if hp == 0:
                    fin = attn_group(0, 0, deque(), per_kt=per_kt_first)
                else:
                    fin = attn_group(hp, 0, qc0_fillers.get(hp, deque()),
                                     finish_prev=fin)
                rem = qc0_fillers.get(hp)
                while rem:
                    rem.popleft()()
            es_z1.close()   # z1T dead after qc0-era fills
            es_wv.close()   # V weight tiles dead
            late = ExitStack()
            latep = late.enter_context(
                tc.tile_pool(name="late", bufs=1, side="right"))
            w2lo_poolA = late.enter_context(
                tc.tile_pool(name="w2loA", bufs=2, side="right"))

            # qc=1 era: first-half proj/LN2 as per-kt fillers; fc1 chunks at
            # group boundaries (gelus in one act-table block); fc2 halves
            # alternate psum pools, oc-major for the w2lo slab.
            mid0 = []
            for tt in range(4):
                mid0.append(lambda tt=tt: proj_fill(qkpsum, tt, 0))
                mid0.append(lambda tt=tt: proj_fill(qkpsum, tt, 1))
                mid0.append(lambda tt=tt: e_stat1(0, tt))
            mid0.append(lambda: e_sqrt4(0))
            mid0 += [lambda tt=tt: e_tr(qkpsum, tt) for tt in range(4)]
            hp_fillers = {0: deque(mid0)}
            f2mid = []
            for oc in range(2):
                for tt in range(4):
                    for half in range(2):
                        for sub in range(2):
                            f2mid.append(
                                lambda tt=tt, oc=oc, half=half, sub=sub,
                                p=(qkpsum if tt % 2 else bcpsum),
                                pt=("qkps" if tt % 2 else "bps"):
                                fc2_sub(ypoolA, p, w2lo_poolA, 0, tt, oc,
                                        half, sub, ps_tag=pt))
            hp_fillers[5] = deque(f2mid[0:11])
            hp_fillers[6] = deque(f2mid[11:22])
            hp_fillers[7] = deque(f2mid[22:32])
            for hp in range(8):
                fin = attn_group(hp, 1, hp_fillers.get(hp, deque()),
                                 finish_prev=fin)
                rem = hp_fillers.get(hp)
                while rem:
                    rem.popleft()()
                if 1 <= hp <= 4:
                    for jt in range(8 * (hp - 1), 8 * hp):
                        fc1_fill(latep, hb_pool, wq_pool, spsum, 0, jt,
                                 ps_tag="sps")
            fin()
            late.close()
        es_kqv.close()  # KT/QT/VP dead

        # ---- tail: second-half proj/LN2/fc1/fc2 ----
        with (
            tc.tile_pool(name="w1pB", bufs=8) as w1_poolB,
            tc.tile_pool(name="hpB", bufs=1) as hpoolB,
            tc.tile_pool(name="hbB", bufs=5) as hb_poolB,
            tc.tile_pool(name="ypB", bufs=3) as ypoolB,
            tc.tile_pool(name="w2loB", bufs=8) as w2lo_poolB,
            tc.tile_pool(name="psT1", bufs=2, space="PSUM") as t1psum,
            tc.tile_pool(name="psT2", bufs=4, space="PSUM") as t2psum,
            tc.tile_pool(name="psTE", bufs=2, space="PSUM") as tepsum,
        ):
            for tt in range(4, 8):
                proj_fill(t2psum, tt, 0, ps_tag="f2t")
                proj_fill(t2psum, tt, 1, ps_tag="f2t")
                e_stat1(4, tt - 4)
            e_sqrt4(4)
            w1_pre = {}
            for jt in range(4):
                w1f = w1_poolB.tile([128, DT, 128], FP8, tag="w1f",
                                    name=f"w1fp{jt}")
                nc.sync.dma_start(out=w1f, in_=w1.ap()[:, jt, :, :])
                w1l = w1_poolB.tile([128, DT, 128], FP8, tag="w1f",
                                    name=f"w1lp{jt}")
                nc.sync.dma_start(out=w1l, in_=w1lo.ap()[:, jt, :, :])
                w1_pre[jt] = (w1f, w1l)
            for tt in range(4, 8):
                e_tr(tepsum, tt, on_act=True)
            for jt in range(32):
                fc1_fill(hpoolB, hb_poolB, w1_poolB, t1psum, 1, jt,
                         ps_tag="qkps", wtag="w1f", pre=w1_pre.get(jt))
            for oc in range(2):
                for tt in range(4):
                    for half in range(2):
                        for sub in range(2):
                            fc2_sub(ypoolB, t2psum, w2lo_poolB, 1, tt, oc,
                                    half, sub, ps_tag="f2t")
        es_ao.close()
        es_keep.close()

    nc.compile()
    return nc


def _kq_perm():
    """new feature order (hgrp, half, hrow, d32) <- orig (4hg+hr)*64+half*32+d."""
    idx = np.empty(1024, np.int64)
    n = 0
    for hg in range(4):
        for hf in range(2):
            for hr in range(4):
                for dd in range(32):
                    idx[n] = (4 * hg + hr) * 64 + hf * 32 + dd
                    n += 1
    return idx


def prep_host_inputs(inputs):
    import ml_dtypes
    f8 = ml_dtypes.float8_e4m3
    bfl = ml_dtypes.bfloat16
    f32 = np.float32

    x = np.asarray(inputs["x"], f32)
    qkv_w = np.asarray(inputs["qkv_w"], f32)
    qkv_b = np.asarray(inputs["qkv_b"], f32)
    proj_w = np.asarray(inputs["proj_w"], f32)
    proj_b = np.asarray(inputs["proj_b"], f32)
    fc1_w = np.asarray(inputs["fc1_w"], f32)
    fc1_b = np.asarray(inputs["fc1_b"], f32)
    fc2_w = np.asarray(inputs["fc2_w"], f32)
    fc2_b = np.asarray(inputs["fc2_b"], f32)
    ln1_w = np.asarray(inputs["ln1_w"], f32)
    ln1_b = np.asarray(inputs["ln1_b"], f32)
    ln2_w = np.asarray(inputs["ln2_w"], f32)
    ln2_b = np.asarray(inputs["ln2_b"], f32)

    wqkv = ln1_w[:, None] * qkv_w          # [1024, 3072]
    bqkv = qkv_b + ln1_b @ qkv_w           # [3072]
    perm = _kq_perm()
    wq = wqkv[:, 0:1024][:, perm] * 32.0
    wk = wqkv[:, 1024:2048][:, perm] * 32.0
    wv = wqkv[:, 2048:3072] * 32.0
    bq_full = np.concatenate([bqkv[0:1024][perm], bqkv[1024:2048][perm]]) * 32.0
    vb = bqkv[2048:3072] * 32.0

    wqkv_all = np.concatenate([wq, wk, wv], axis=1)      # [1024, 3072]
    wqkv_dev = np.ascontiguousarray(
        wqkv_all.reshape(8, 128, 24, 128).transpose(1, 2, 0, 3)).astype(f8)
    bq_dev = np.ascontiguousarray(bq_full.reshape(16, 128).T).astype(f32)

    wproj_dev = np.ascontiguousarray(
        (proj_w * 32.0).reshape(8, 128, 1024).transpose(1, 0, 2)).astype(f8)

    w1s = ln2_w[:, None] * fc1_w * 32.0
    w1q = w1s.astype(f8)
    w1r = (w1s - w1q.astype(f32)).astype(f8)   # subnormal residual
    b1_dev = (fc1_b + ln2_b @ fc1_w).astype(f32)
    w1_dev = np.ascontiguousarray(
        w1q.astype(f32).reshape(8, 128, 32, 128).transpose(1, 2, 0, 3)).astype(f8)
    w1lo_dev = np.ascontiguousarray(
        w1r.astype(f32).reshape(8, 128, 32, 128).transpose(1, 2, 0, 3)).astype(f8)

    w2s = fc2_w * 64.0
    w2q = w2s.astype(f8)
    w2r = (w2s - w2q.astype(f32)).astype(f8)
    w2_dev = np.ascontiguousarray(
        w2q.astype(f32).reshape(32, 128, 1024).transpose(1, 0, 2)).astype(f8)
    w2lo_dev = np.ascontiguousarray(
        w2r.astype(f32).reshape(32, 128, 1024).transpose(1, 0, 2)).astype(f8)

    shared = {
        "wqkv": wqkv_dev, "bq": bq_dev, "vbias": vb.astype(bfl),
        "wproj": wproj_dev, "bproj": (proj_b * 1024.0).astype(bfl),
        "w1": w1_dev, "w1lo": w1lo_dev, "b1": b1_dev,
        "w2": w2_dev, "w2lo": w2lo_dev, "b2": (fc2_b * 64.0).astype(bfl),
    }
    in_maps = []
    for c in range(N_CORES):
        b, half = c // 2, c % 2
        own = x[b, half * 1024:(half + 1) * 1024]
        other = x[b, (1 - half) * 1024:(2 - half) * 1024]
        xc = np.concatenate([own, other], axis=0).astype(bfl)
        in_maps.append({"x": np.ascontiguousarray(xc), **shared})
    return in_maps


# ---------------------------------------------------------------------------
# Cached PJRT runner (jit once, reuse across kernel() calls)
# ---------------------------------------------------------------------------
_CACHE = {}


def _get_runner():
    if "runner" in _CACHE:
        return _CACHE["runner"]

    from jax.experimental.shard_map import shard_map
    from jax.sharding import Mesh, PartitionSpec
    from concourse.bass2jax import (
        _bass_exec_p, install_neuronx_cc_hook, partition_id_tensor,
    )

    nc = build_nc()
    install_neuronx_cc_hook()

    partition_name = nc.partition_id_tensor.name if nc.partition_id_tensor else None
    in_names, out_names, out_avals, zero_shapes = [], [], [], []
    for alloc in nc.m.functions[0].allocations:
        if not isinstance(alloc, mybir.MemoryLocationSet):
            continue
        name = alloc.memorylocations[0].name
        if alloc.kind == "ExternalInput":
            if name != partition_name:
                in_names.append(name)
        elif alloc.kind == "ExternalOutput":
            shape = tuple(alloc.tensor_shape)
            dtype = mybir.dt.np(alloc.dtype)
            out_names.append(name)
            out_avals.append(jax.core.ShapedArray(shape, dtype))
            zero_shapes.append((shape, dtype))
    n_params = len(in_names)
    n_outs = len(out_names)
    all_in = list(in_names) + list(out_names)
    if partition_name is not None:
        all_in.append(partition_name)
    donate = tuple(range(n_params, n_params + n_outs))

    def _body(*args):
        operands = list(args)
        if partition_name is not None:
            operands.append(partition_id_tensor())
        outs = _bass_exec_p.bind(
            *operands,
            out_avals=tuple(out_avals),
            in_names=tuple(all_in),
            out_names=tuple(out_names),
            lowering_input_output_aliases=(),
            sim_require_finite=True,
            sim_require_nnan=True,
            nc=nc,
        )
        return tuple(outs)

    devices = jax.devices()[:N_CORES]
    mesh = Mesh(np.asarray(devices), ("core",))
    sharded = jax.jit(
        shard_map(
            _body, mesh=mesh,
            in_specs=(PartitionSpec("core"),) * (n_params + n_outs),
            out_specs=(PartitionSpec("core"),) * n_outs,
            check_rep=False,
        ),
        donate_argnums=donate, keep_unused=True,
    )

    def run(in_maps):
        concat_in = [
            np.concatenate([np.asarray(m[name]) for m in in_maps], axis=0)
            for name in in_names
        ]
        concat_zeros = [
            np.zeros((N_CORES * s[0], *s[1:]), dt) for (s, dt) in zero_shapes
        ]
        out_arrs = sharded(*concat_in, *concat_zeros)
        per_core = []
        for c in range(N_CORES):
            per_core.append({
                name: np.asarray(out_arrs[i]).reshape(
                    N_CORES, *out_avals[i].shape)[c]
                for i, name in enumerate(out_names)
            })
        return per_core

    _CACHE["runner"] = run
    return run


def kernel(**inputs) -> np.ndarray:
    run = _get_runner()
    in_maps = prep_host_inputs(inputs)
    results = run(in_maps)
    out = np.zeros((4, 2048, 1024), np.float32)
    for c in range(N_CORES):
        b, half = c // 2, c % 2
        out[b, half * 1024:(half + 1) * 1024] = results[c]["y"]
    return out


# revision 18
# speedup vs baseline: 1.0003x; 1.0003x over previous
"""Transformer block (pre-norm attention + MLP) on 8 TRN2 NeuronCores — fp8 DR
with subnormal-residual error compensation.

Sharding: 8 cores = 4 batch x 2 seq halves (data parallel, no collectives).
Each core computes 1024 own query tokens; K/V built for all 2048 tokens of its
batch element (own tokens first, permutation-invariant softmax).

All big GEMMs run fp8e4m3 with DoubleRow perf mode (contraction pairs of 128
partitions per instruction). Weights are scaled by powers of two host-side so
fp8 operands sit near unit scale; inverse scales fold into activation scale
params / epilogue multiplies (exact exponent shifts). Softmax: exp(s/8-4ln2)
keeps ex in [0,16) for fp8; denominators come from a ones-column in V; the
/16 cancels in normalization.

The MLP carries error compensation: each quantized operand Q(a) is paired
with a subnormal-fp8 residual (a - Q(a)), and the matmuls accumulate
main + residual terms (z2, w1, w2, and the gelu output h). This cuts the
final error ~4.5x vs plain fp8 while staying on the 2x DoubleRow path.

Schedule: query-chunk-major attention; first-half proj/LN2/fc1/fc2 runs as
filler work inside the second attention half (Act exp stream = critical
path). QKV/V fills are JIT inside the first group; LN1 is split around the
first fills so attention starts early.
"""

from collections import deque
from contextlib import ExitStack

import numpy as np

try:
    import jax
    jax.config.update("jax_compilation_cache_dir", "/tmp/jax_bass_cache")
    jax.config.update("jax_persistent_cache_min_compile_time_secs", 0.0)
    jax.config.update("jax_persistent_cache_min_entry_size_bytes", -1)
except Exception:
    import jax

import concourse.bacc as bacc
import concourse.bass as bass
import concourse.mybir as mybir
import concourse.tile as tile
from concourse.masks import make_identity

FP32 = mybir.dt.float32
BF16 = mybir.dt.bfloat16
FP8 = mybir.dt.float8e4
AF = mybir.ActivationFunctionType
ALU = mybir.AluOpType
DR = mybir.MatmulPerfMode.DoubleRow

D = 1024
DT = 8
H = 16
HD = 64
HID = 4096
T_ALL = 2048
T_OWN = 1024
EPS = 1e-6
N_CORES = 8
NEG4LN2 = -2.772588722239781


def build_nc():
    nc = bacc.Bacc("TRN2", target_bir_lowering=False, debug=False,
                   num_devices=N_CORES)

    x = nc.dram_tensor("x", [T_ALL, D], BF16, kind="ExternalInput")
    # [128, 24 fb, 8 dt, 128] : fb 0-7 Q (perm), 8-15 K (perm), 16-23 V
    wqkv = nc.dram_tensor("wqkv", [128, 24, DT, 128], FP8, kind="ExternalInput")
    bq = nc.dram_tensor("bq", [128, 16], FP32, kind="ExternalInput")
    vbias = nc.dram_tensor("vbias", [D], BF16, kind="ExternalInput")
    wproj = nc.dram_tensor("wproj", [128, DT, D], FP8, kind="ExternalInput")
    bproj = nc.dram_tensor("bproj", [D], BF16, kind="ExternalInput")
    w1 = nc.dram_tensor("w1", [128, 32, DT, 128], FP8, kind="ExternalInput")
    w1lo = nc.dram_tensor("w1lo", [128, 32, DT, 128], FP8, kind="ExternalInput")
    b1 = nc.dram_tensor("b1", [HID], FP32, kind="ExternalInput")
    w2 = nc.dram_tensor("w2", [128, 32, D], FP8, kind="ExternalInput")
    w2lo = nc.dram_tensor("w2lo", [128, 32, D], FP8, kind="ExternalInput")
    b2 = nc.dram_tensor("b2", [D], BF16, kind="ExternalInput")
    y = nc.dram_tensor("y", [T_OWN, D], FP32, kind="ExternalOutput")

    with tile.TileContext(nc) as tc, ExitStack() as ctx:
        P = ctx.enter_context

        singles = P(tc.tile_pool(name="singles", bufs=1))
        statpool = P(tc.tile_pool(name="stat", bufs=3))
        znpool = P(tc.tile_pool(name="zn", bufs=2))
        xo_pool = P(tc.tile_pool(name="xop", bufs=3))
        es_keep = ExitStack()
        keepp = es_keep.enter_context(
            tc.tile_pool(name="keep", bufs=1, side="right"))

        # ---- constants ----
        ident = singles.tile([128, 128], BF16)
        make_identity(nc, ident)
        ones64 = singles.tile([1, 64], BF16)
        nc.vector.memset(ones64, 1.0)
        ones_bf = singles.tile([1, 128], BF16)
        nc.vector.memset(ones_bf, 1.0)
        eps_sb = singles.tile([128, 1], FP32)
        nc.vector.memset(eps_sb, EPS)
        ebias = singles.tile([128, 1], FP32)
        nc.vector.memset(ebias, NEG4LN2)
        bq_sb = singles.tile([128, 16], FP32)
        nc.sync.dma_start(out=bq_sb, in_=bq.ap())
        b1_sb = singles.tile([128, 32], FP32)
        nc.sync.dma_start(out=b1_sb, in_=b1.ap().rearrange("(f p) -> p f", p=128))
        bproj_sb = singles.tile([1, D], BF16)
        nc.sync.dma_start(out=bproj_sb, in_=bproj.ap().rearrange("(o f) -> o f", o=1))
        b2_sb = singles.tile([1, D], BF16)
        nc.sync.dma_start(out=b2_sb, in_=b2.ap().rearrange("(o f) -> o f", o=1))
        vbias_sb = singles.tile([128, D], BF16)
        nc.sync.dma_start(
            out=vbias_sb,
            in_=bass.AP(tensor=vbias, offset=0, ap=[[0, 128], [1, D]]),
        )

        # ---- long-lived tiles ----
        projw_sb = keepp.tile([128, DT, D], FP8, tag="projw")
        w2_sb = keepp.tile([128, 32, D], FP8, tag="w2sb")
        x2_all = keepp.tile([128, DT, D], BF16, tag="x2")
        z2T = keepp.tile([128, DT, T_OWN], FP8, tag="z2T")
        z2loT = keepp.tile([128, DT, T_OWN], FP8, tag="z2loT")

        es_ao = ExitStack()
        aop = es_ao.enter_context(tc.tile_pool(name="aop", bufs=1, side="right"))
        aoT = aop.tile([128, DT, T_OWN], FP8, tag="aoT")

        x_t = x.ap().rearrange("(t p) f -> p t f", p=128)

        es_kqv = ExitStack()
        kqvp = es_kqv.enter_context(tc.tile_pool(name="kqvp", bufs=1))
        KT = kqvp.tile([128, 4, 2, T_ALL], FP8, tag="kt")     # hgrp, half, tok
        QT = kqvp.tile([128, 4, 2, T_OWN], FP8, tag="qt")
        VP = kqvp.tile([128, 16, 16, 80], FP8, tag="vp")      # kt, head, hd|1
        vp_ones = VP[:, :, :, 64:65]
        nc.vector.memset(vp_ones, 1.0)

        wq_state, wv_state, h_tiles, e_state, fc2_ps = {}, {}, {}, {}, {}

        def ln_stats(xt):
            stats = statpool.tile([128, 2, 6], FP32, tag="stats")
            nc.vector.bn_stats(out=stats[:, 0, :], in_=xt[:, 0:512])
            nc.vector.bn_stats(out=stats[:, 1, :], in_=xt[:, 512:1024])
            mv = statpool.tile([128, 2], FP32, tag="mv")
            nc.vector.bn_aggr(out=mv, in_=stats)
            sd = statpool.tile([128, 1], FP32, tag="sd")
            nc.scalar.activation(out=sd, in_=mv[:, 1:2], func=AF.Sqrt,
                                 bias=eps_sb)
            rinv = statpool.tile([128, 1], FP32, tag="rinv")
            nc.vector.reciprocal(out=rinv, in_=sd)
            return mv, rinv

        def ln1_tile(trp, tt, z1T, on_act):
            """LN1 one 128-token tile of x (streamed) into z1T fp8."""
            xt = xo_pool.tile([128, D], BF16, tag="xo")
            nc.sync.dma_start(out=xt, in_=x_t[:, tt, :])
            mv, rinv = ln_stats(xt)
            zn = znpool.tile([128, D], BF16, tag="zn")
            nc.gpsimd.tensor_scalar(
                out=zn, in0=xt, scalar1=mv[:, 0:1], scalar2=rinv,
                op0=ALU.subtract, op1=ALU.mult,
            )
            for g in range(2):
                ps = trp.tile([128, 4, 128], BF16, tag="sps")
                for i in range(4):
                    nc.tensor.transpose(
                        ps[:, i, :],
                        zn[:, (4 * g + i) * 128:(4 * g + i + 1) * 128], ident)
                dst = z1T[:, 4 * g:4 * g + 4, tt * 128:(tt + 1) * 128]
                if on_act:
                    nc.scalar.copy(out=dst, in_=ps)
                else:
                    nc.vector.tensor_copy(out=dst, in_=ps)

        def kq_fill(wq_pool, psp, z1T, fb, hg, hf, dst, tcn):
            """One 512-token chunk of one K/Q feature block."""
            if fb not in wq_state:
                wq_f = wq_pool.tile([128, DT, 128], FP8, tag="wqf")
                nc.sync.dma_start(out=wq_f, in_=wqkv.ap()[:, fb, :, :])
                wq_state[fb] = wq_f
            wq_f = wq_state[fb]
            ps = psp.tile([128, 512], FP32, tag="qkps")
            for t in range(4):
                nc.tensor.matmul(
                    ps, wq_f[:, 2 * t:2 * t + 2, :],
                    z1T[:, 2 * t:2 * t + 2, tcn * 512:(tcn + 1) * 512],
                    start=(t == 0), stop=(t == 3), perf_mode=DR,
                )
            nc.vector.tensor_scalar(
                out=dst[:, hg, hf, tcn * 512:(tcn + 1) * 512],
                in0=ps, scalar1=bq_sb[:, fb:fb + 1],
                scalar2=None, op0=ALU.add,
            )

        def v_fill(wv_pool, psp, z1T, vc, tt):
            if vc not in wv_state:
                wv = wv_pool.tile([128, DT, 4, 128], FP8, tag="wvf")
                for b in range(4):
                    nc.sync.dma_start(
                        out=wv[:, :, b, :],
                        in_=wqkv.ap()[:, 16 + 4 * vc + b, :, :])
                wv_state[vc] = wv
            wv = wv_state[vc]
            ps = psp.tile([128, 512], FP32, tag="qkps")
            for t in range(4):
                nc.tensor.matmul(
                    ps, z1T[:, 2 * t:2 * t + 2, tt * 128:(tt + 1) * 128],
                    wv[:, 2 * t:2 * t + 2, :, :].rearrange("p d b f -> p d (b f)"),
                    start=(t == 0), stop=(t == 3), perf_mode=DR,
                )
            dst = VP[:, tt, vc * 8:(vc + 1) * 8, 0:64]
            srcp = ps.rearrange("p (h e) -> p h e", e=HD)
            vb = vbias_sb[:, vc * 512:(vc + 1) * 512].rearrange(
                "p (h e) -> p h e", e=HD)
            nc.vector.scalar_tensor_tensor(
                out=dst, in0=srcp, scalar=0.0, in1=vb,
                op0=ALU.bypass, op1=ALU.add,
            )

        def proj_fill(psp, tt, oc, ps_tag="qkps"):
            ps = psp.tile([128, 512], FP32, tag=ps_tag)
            for t in range(4):
                nc.tensor.matmul(
                    ps, aoT[:, 2 * t:2 * t + 2, tt * 128:(tt + 1) * 128],
                    projw_sb[:, 2 * t:2 * t + 2, oc * 512:(oc + 1) * 512],
                    start=(t == 0), stop=False, perf_mode=DR,
                )
            nc.tensor.matmul(
                ps, ones_bf, bproj_sb[:, oc * 512:(oc + 1) * 512],
                start=False, stop=True,
            )
            xt = xo_pool.tile([128, 512], BF16, tag="xo", name=f"xr{tt}_{oc}")
            nc.sync.dma_start(
                out=xt,
                in_=x_t[:, tt, oc * 512:(oc + 1) * 512])
            nc.vector.scalar_tensor_tensor(
                out=x2_all[:, tt, oc * 512:(oc + 1) * 512],
                in0=ps, scalar=2.0 ** -10, in1=xt,
                op0=ALU.mult, op1=ALU.add,
            )

        def e_stat1(base, i):
            if base not in e_state:
                mv4 = statpool.tile([128, 4, 2], FP32, tag="mv4",
                                    name=f"mv4_{base}")
                e_state[base] = [mv4, None]
            mv4 = e_state[base][0]
            stats = statpool.tile([128, 2, 6], FP32, tag="stats")
            xt = x2_all[:, base + i, :]
            nc.vector.bn_stats(out=stats[:, 0, :], in_=xt[:, 0:512])
            nc.vector.bn_stats(out=stats[:, 1, :], in_=xt[:, 512:1024])
            nc.vector.bn_aggr(out=mv4[:, i, :], in_=stats)

        def e_sqrt4(base):
            mv4 = e_state[base][0]
            sd4 = statpool.tile([128, 4], FP32, tag="sd4", name=f"sd4_{base}")
            nc.scalar.activation(out=sd4, in_=mv4[:, :, 1], func=AF.Sqrt,
                                 bias=eps_sb)
            rinv4 = statpool.tile([128, 4], FP32, tag="rinv4",
                                  name=f"rinv4_{base}")
            nc.vector.reciprocal(out=rinv4, in_=sd4)
            e_state[base][1] = rinv4

        def e_tr(psp, tt, ps_tag="qkps", on_act=False):
            """LN2 apply + transpose for one tile -> z2T fp8 + z2loT resid."""
            base = tt // 4 * 4
            mv4, rinv4 = e_state[base]
            i = tt - base
            zn = znpool.tile([128, D], BF16, tag="zn")
            nc.gpsimd.tensor_scalar(
                out=zn, in0=x2_all[:, tt, :], scalar1=mv4[:, i, 0:1],
                scalar2=rinv4[:, i:i + 1], op0=ALU.subtract, op1=ALU.mult,
            )
            for g in range(2):
                ps = psp.tile([128, 4, 128], BF16, tag=ps_tag)
                for i2 in range(4):
                    nc.tensor.transpose(
                        ps[:, i2, :],
                        zn[:, (4 * g + i2) * 128:(4 * g + i2 + 1) * 128], ident)
                d8 = z2T[:, 4 * g:4 * g + 4, tt * 128:(tt + 1) * 128]
                if on_act:
                    nc.scalar.copy(out=d8, in_=ps)
                else:
                    nc.vector.tensor_copy(out=d8, in_=ps)
                dlo = z2loT[:, 4 * g:4 * g + 4, tt * 128:(tt + 1) * 128]
                nc.vector.tensor_tensor(out=dlo, in0=ps, in1=d8,
                                        op=ALU.subtract)

        def fc1_fill(hpool, hb_pool, w1_pool, psp, tc2, jt, ps_tag="qkps",
                     wtag="wqf", pre=None):
            """fc1 with z2 + w1 compensation, gelu, then h8/hlo split on
            gpsimd (subnormal residual)."""
            if tc2 not in h_tiles:
                h8 = hpool.tile([128, 32, 512], FP8, tag="h8",
                                name=f"h8_{tc2}")
                hlo = hpool.tile([128, 32, 512], FP8, tag="hlo",
                                 name=f"hlo_{tc2}")
                h_tiles[tc2] = (h8, hlo)
            h8, hlo = h_tiles[tc2]
            if pre is not None:
                w1f, w1l = pre
            else:
                w1f = w1_pool.tile([128, DT, 128], FP8, tag=wtag,
                                   name=f"w1f_{tc2}_{jt}")
                nc.sync.dma_start(out=w1f, in_=w1.ap()[:, jt, :, :])
                w1l = w1_pool.tile([128, DT, 128], FP8, tag=wtag,
                                   name=f"w1l_{tc2}_{jt}")
                nc.sync.dma_start(out=w1l, in_=w1lo.ap()[:, jt, :, :])
            ps = psp.tile([128, 512], FP32, tag=ps_tag)
            sl = slice(tc2 * 512, (tc2 + 1) * 512)
            for t in range(4):
                nc.tensor.matmul(
                    ps, w1f[:, 2 * t:2 * t + 2, :],
                    z2T[:, 2 * t:2 * t + 2, sl],
                    start=(t == 0), stop=False, perf_mode=DR,
                )
            for t in range(4):
                nc.tensor.matmul(
                    ps, w1f[:, 2 * t:2 * t + 2, :],
                    z2loT[:, 2 * t:2 * t + 2, sl],
                    start=False, stop=False, perf_mode=DR,
                )
            for t in range(4):
                nc.tensor.matmul(
                    ps, w1l[:, 2 * t:2 * t + 2, :],
                    z2T[:, 2 * t:2 * t + 2, sl],
                    start=False, stop=(t == 3), perf_mode=DR,
                )
            hb = hb_pool.tile([128, 512], BF16, tag="hb")
            nc.scalar.activation(
                out=hb, in_=ps, func=AF.Gelu,
                scale=2.0 ** -5, bias=b1_sb[:, jt:jt + 1],
            )
            nc.vector.tensor_copy(out=h8[:, jt, :], in_=hb)
            nc.gpsimd.tensor_tensor(out=hlo[:, jt, :], in0=hb,
                                    in1=h8[:, jt, :], op=ALU.subtract)

        def fc2_sub(ypool, psp, w2lo_pool, tc2, tt, oc, half, sub,
                    ps_tag="qkps"):
            """Quarter of one fc2 chunk (4 pairs, 12 DR) with h and w2
            compensation; streams one 8-row w2-residual tile."""
            h8, hlo = h_tiles[tc2]
            tglob = tc2 * 4 + tt
            key = (tc2, tt, oc)
            if half == 0 and sub == 0:
                fc2_ps[key] = psp.tile([128, 512], FP32, tag=ps_tag,
                                       name=f"f2_{tc2}_{tt}_{oc}")
            ps = fc2_ps[key]
            tsl = slice(tt * 128, (tt + 1) * 128)
            osl = slice(oc * 512, (oc + 1) * 512)
            base = 16 * half + 8 * sub
            w2l = w2lo_pool.tile([128, 8, 512], FP8, tag="w2lo",
                                 name=f"w2l_{tc2}_{tt}_{oc}_{half}_{sub}")
            nc.sync.dma_start(out=w2l, in_=w2lo.ap()[:, base:base + 8, osl])
            for p in range(4):
                jt = 8 * half + 4 * sub + p
                jsl = slice(2 * jt, 2 * jt + 2)
                lsl = slice(2 * p, 2 * p + 2)
                nc.tensor.matmul(ps, h8[:, jsl, tsl], w2_sb[:, jsl, osl],
                                 start=(jt == 0), stop=False, perf_mode=DR)
                nc.tensor.matmul(ps, hlo[:, jsl, tsl], w2_sb[:, jsl, osl],
                                 start=False, stop=False, perf_mode=DR)
                nc.tensor.matmul(ps, h8[:, jsl, tsl], w2l[:, lsl, :],
                                 start=False, stop=False, perf_mode=DR)
            if not (half == 1 and sub == 1):
                return
            nc.tensor.matmul(
                ps, ones_bf, b2_sb[:, osl], start=False, stop=True,
            )
            ys = ypool.tile([128, 512], FP32, tag="ys")
            nc.vector.scalar_tensor_tensor(
                out=ys, in0=ps, scalar=2.0 ** -6,
                in1=x2_all[:, tglob, osl],
                op0=ALU.mult, op1=ALU.add,
            )
            nc.sync.dma_start(
                out=y[tglob * 128:(tglob + 1) * 128, osl], in_=ys,
            )

        # ---- attention + interleaved everything ----
        with (
            tc.tile_pool(name="wq", bufs=4, side="right") as wq_pool,
            tc.tile_pool(name="hbA", bufs=5) as hb_pool,
            tc.tile_pool(name="ypA", bufs=1) as ypoolA,
            tc.tile_pool(name="psB", bufs=1, space="PSUM") as qkpsum,
            tc.tile_pool(name="exs", bufs=1) as exp_pool,
            tc.tile_pool(name="nrm", bufs=2) as nrm_pool,
            tc.tile_pool(name="psS", bufs=2, space="PSUM") as spsum,
            tc.tile_pool(name="psAV", bufs=2, space="PSUM") as avpsum,
            tc.tile_pool(name="psBC", bufs=1, space="PSUM") as bcpsum,
        ):
            es_wv = ExitStack()
            wv_pool = es_wv.enter_context(
                tc.tile_pool(name="wv", bufs=2, side="right"))
            es_z1 = ExitStack()
            z1p = es_z1.enter_context(
                tc.tile_pool(name="z1p", bufs=1, side="right"))
            z1T = z1p.tile([128, DT, T_ALL], FP8, tag="z1T")

            def attn_group(hp, qc, fillers, per_kt=None, finish_prev=None):
                """Head pair (2hp, 2hp+1), query chunk qc. The previous
                group's last AV pair + normalization run deferred at kt==0
                so they don't gate this group's first scores/exp."""
                h0, h1 = 2 * hp, 2 * hp + 1
                hg = hp // 2
                EX4 = exp_pool.tile([128, 2, 4, 512], FP8, tag="ex4")
                avs = []
                for _i in range(2):
                    av_t = avpsum.tile([65, 512], FP32, tag="av")
                    avs.append(av_t)
                for kt in range(16):
                    sp = spsum.tile([128, 2, 512], FP32, tag="sps")
                    for i, h in enumerate((h0, h1)):
                        hr = h % 4
                        nc.tensor.matmul(
                            sp[:, i, :],
                            KT[32 * hr:32 * hr + 32, hg, :,
                               kt * 128:(kt + 1) * 128],
                            QT[32 * hr:32 * hr + 32, hg, :,
                               qc * 512:(qc + 1) * 512],
                            start=True, stop=True, perf_mode=DR,
                            tile_position=(32 * hr, 0),
                        )
                    nc.scalar.activation(
                        out=EX4[:, :, kt % 4, :], in_=sp, func=AF.Exp,
                        scale=2.0 ** -13, bias=ebias)
                    if kt == 0 and finish_prev is not None:
                        finish_prev()
                    if per_kt is not None:
                        per_kt(kt)
                    if kt % 2 == 1 and kt < 15:
                        s0 = 2 * ((kt // 2) % 2)
                        for i, h in enumerate((h0, h1)):
                            nc.tensor.matmul(
                                avs[i], VP[:, kt - 1:kt + 1, h, 0:65],
                                EX4[:, i, s0:s0 + 2, :],
                                start=(kt == 1), stop=False,
                                perf_mode=DR,
                            )
                    if fillers:
                        fillers.popleft()()

                def finish():
                    for i, h in enumerate((h0, h1)):
                        nc.tensor.matmul(
                            avs[i], VP[:, 14:16, h, 0:65],
                            EX4[:, i, 2:4, :],
                            start=False, stop=True, perf_mode=DR,
                        )
                    for i, h in enumerate((h0, h1)):
                        av = avs[i]
                        rec = nrm_pool.tile([1, 512], BF16, tag="rec")
                        with nc.allow_low_precision(reason="softmax denom"):
                            nc.vector.reciprocal(out=rec, in_=av[64:65, :])
                        bps = bcpsum.tile([64, 512], FP32, tag="bps")
                        nc.tensor.matmul(bps, ones64, rec, start=True,
                                         stop=True)
                        bcs = nrm_pool.tile([64, 512], BF16, tag="bcs")
                        nc.vector.tensor_copy(out=bcs, in_=bps)
                        pr, ft = (h % 2) * 64, h // 2
                        nc.vector.scalar_tensor_tensor(
                            out=aoT[pr:pr + 64, ft, qc * 512:(qc + 1) * 512],
                            in0=av[0:64, :], scalar=0.0, in1=bcs,
                            op0=ALU.bypass, op1=ALU.mult,
                        )
                return finish

            def kqf(hg, hf, tcn, q=False):
                fb = (2 * hg + hf) if q else (8 + 2 * hg + hf)
                dst = QT if q else KT
                return lambda: kq_fill(wq_pool, qkpsum, z1T, fb, hg, hf,
                                       dst, tcn)

            def vf(vc, tt):
                return lambda: v_fill(wv_pool, qkpsum, z1T, vc, tt)

            # phase A: LN1 tiles (drain on Act: it is idle here)
            for tt in range(16):
                ln1_tile(spsum, tt, z1T, on_act=True)
            kqf(0, 0, 0)(); kqf(0, 1, 0)()
            kqf(0, 0, 0, q=True)(); kqf(0, 1, 0, q=True)()
            vf(0, 0)()
            nc.sync.dma_start(out=projw_sb, in_=wproj.ap())
            nc.sync.dma_start(out=w2_sb, in_=w2.ap())

            def per_kt_first(kt):
                jit = {0: kqf(0, 0, 1), 1: kqf(0, 1, 1),
                       2: kqf(0, 0, 2), 3: kqf(0, 1, 2),
                       4: kqf(0, 0, 1, q=True), 5: kqf(0, 1, 1, q=True),
                       6: kqf(0, 0, 3), 7: kqf(0, 1, 3)}
                if kt in jit:
                    jit[kt]()
                if kt < 15:
                    vf(0, kt + 1)()

            qc0_fillers = {
                1: deque([kqf(1, hf, tcn) for hf in range(2) for tcn in range(4)]
                         + [kqf(1, hf, tcn, q=True) for hf in range(2)
                            for tcn in range(2)]),
                2: deque([vf(1, tt) for tt in range(8)]
                         + [kqf(2, hf, tcn) for hf in range(2) for tcn in (0, 1)]),
                3: deque([vf(1, tt) for tt in range(8, 16)]
                         + [kqf(2, hf, tcn) for hf in range(2) for tcn in (2, 3)]
                         + [kqf(2, hf, tcn, q=True) for hf in range(2)
                            for tcn in range(2)]),
                4: deque([kqf(3, hf, tcn) for hf in range(2) for tcn in range(4)]
                         + [kqf(3, hf, tcn, q=True) for hf in range(2)
                            for tcn in range(2)]),
            }
            fin = None
            for hp in range(8):
                if hp == 0:
                    fin = attn_group(0, 0, deque(), per_kt=per_kt_first)
                else:
                    fin = attn_group(hp, 0, qc0_fillers.get(hp, deque()),
                                     finish_prev=fin)
                rem = qc0_fillers.get(hp)
                while rem:
                    rem.popleft()()
            es_z1.close()   # z1T dead after qc0-era fills
            es_wv.close()   # V weight tiles dead
            late = ExitStack()
            latep = late.enter_context(
                tc.tile_pool(name="late", bufs=1, side="right"))
            w2lo_poolA = late.enter_context(
                tc.tile_pool(name="w2loA", bufs=2, side="right"))

            # qc=1 era: first-half proj/LN2 as per-kt fillers; fc1 chunks at
            # group boundaries (gelus in one act-table block); fc2 halves
            # alternate psum pools, oc-major for the w2lo slab.
            mid0 = []
            for tt in range(4):
                mid0.append(lambda tt=tt: proj_fill(qkpsum, tt, 0))
                mid0.append(lambda tt=tt: proj_fill(qkpsum, tt, 1))
                mid0.append(lambda tt=tt: e_stat1(0, tt))
            mid0.append(lambda: e_sqrt4(0))
            mid0 += [lambda tt=tt: e_tr(qkpsum, tt) for tt in range(4)]
            hp_fillers = {0: deque(mid0)}
            f2mid = []
            for oc in range(2):
                for tt in range(4):
                    for half in range(2):
                        for sub in range(2):
                            f2mid.append(
                                lambda tt=tt, oc=oc, half=half, sub=sub,
                                p=(qkpsum if tt % 2 else bcpsum),
                                pt=("qkps" if tt % 2 else "bps"):
                                fc2_sub(ypoolA, p, w2lo_poolA, 0, tt, oc,
                                        half, sub, ps_tag=pt))
            hp_fillers[5] = deque(f2mid[0:11])
            hp_fillers[6] = deque(f2mid[11:22])
            hp_fillers[7] = deque(f2mid[22:32])
            for hp in range(8):
                fin = attn_group(hp, 1, hp_fillers.get(hp, deque()),
                                 finish_prev=fin)
                rem = hp_fillers.get(hp)
                while rem:
                    rem.popleft()()
                if 1 <= hp <= 4:
                    for jt in range(8 * (hp - 1), 8 * hp):
                        fc1_fill(latep, hb_pool, wq_pool, spsum, 0, jt,
                                 ps_tag="sps")
            fin()
            late.close()
        es_kqv.close()  # KT/QT/VP dead

        # ---- tail: second-half proj/LN2/fc1/fc2 ----
        with (
            tc.tile_pool(name="w1pB", bufs=8) as w1_poolB,
            tc.tile_pool(name="hpB", bufs=1) as hpoolB,
            tc.tile_pool(name="hbB", bufs=5) as hb_poolB,
            tc.tile_pool(name="ypB", bufs=3) as ypoolB,
            tc.tile_pool(name="w2loB", bufs=8) as w2lo_poolB,
            tc.tile_pool(name="psT1", bufs=2, space="PSUM") as t1psum,
            tc.tile_pool(name="psT2", bufs=4, space="PSUM") as t2psum,
            tc.tile_pool(name="psTE", bufs=2, space="PSUM") as tepsum,
        ):
            for tt in range(4, 8):
                proj_fill(t2psum, tt, 0, ps_tag="f2t")
                proj_fill(t2psum, tt, 1, ps_tag="f2t")
                e_stat1(4, tt - 4)
            e_sqrt4(4)
            w1_pre = {}
            for jt in range(4):
                w1f = w1_poolB.tile([128, DT, 128], FP8, tag="w1f",
                                    name=f"w1fp{jt}")
                nc.sync.dma_start(out=w1f, in_=w1.ap()[:, jt, :, :])
                w1l = w1_poolB.tile([128, DT, 128], FP8, tag="w1f",
                                    name=f"w1lp{jt}")
                nc.sync.dma_start(out=w1l, in_=w1lo.ap()[:, jt, :, :])
                w1_pre[jt] = (w1f, w1l)
            for tt in range(4, 8):
                e_tr(tepsum, tt, on_act=True)
            for jt in range(32):
                fc1_fill(hpoolB, hb_poolB, w1_poolB, t1psum, 1, jt,
                         ps_tag="qkps", wtag="w1f", pre=w1_pre.get(jt))
            # tail fc2: load each w2-residual slice ONCE and share it across
            # all four token chunks (4 psums open in t2psum) — avoids 4x
            # redundant w2lo DMA traffic in the PE+DMA-bound drain.
            for oc in range(2):
                osl = slice(oc * 512, (oc + 1) * 512)
                pss = []
                for tt in range(4):
                    ps_t = t2psum.tile([128, 512], FP32, tag="f2t",
                                       name=f"f2B_{tt}_{oc}")
                    pss.append(ps_t)
                for half in range(2):
                    for sub in range(2):
                        base = 16 * half + 8 * sub
                        w2l = w2lo_poolB.tile([128, 8, 512], FP8, tag="w2lo",
                                              name=f"w2lB_{oc}_{half}_{sub}")
                        nc.sync.dma_start(
                            out=w2l, in_=w2lo.ap()[:, base:base + 8, osl])
                        h8, hlo = h_tiles[1]
                        for tt in range(4):
                            tsl = slice(tt * 128, (tt + 1) * 128)
                            for p in range(4):
                                jt = 8 * half + 4 * sub + p
                                jsl = slice(2 * jt, 2 * jt + 2)
                                lsl = slice(2 * p, 2 * p + 2)
                                nc.tensor.matmul(
                                    pss[tt], h8[:, jsl, tsl],
                                    w2_sb[:, jsl, osl],
                                    start=(jt == 0), stop=False, perf_mode=DR)
                                nc.tensor.matmul(
                                    pss[tt], hlo[:, jsl, tsl],
                                    w2_sb[:, jsl, osl],
                                    start=False, stop=False, perf_mode=DR)
                                nc.tensor.matmul(
                                    pss[tt], h8[:, jsl, tsl], w2l[:, lsl, :],
                                    start=False, stop=False, perf_mode=DR)
                for tt in range(4):
                    tglob = 4 + tt
                    nc.tensor.matmul(
                        pss[tt], ones_bf, b2_sb[:, osl],
                        start=False, stop=True,
                    )
                    ys = ypoolB.tile([128, 512], FP32, tag="ys")
                    nc.vector.scalar_tensor_tensor(
                        out=ys, in0=pss[tt], scalar=2.0 ** -6,
                        in1=x2_all[:, tglob, osl],
                        op0=ALU.mult, op1=ALU.add,
                    )
                    nc.sync.dma_start(
                        out=y[tglob * 128:(tglob + 1) * 128, osl], in_=ys,
                    )
        es_ao.close()
        es_keep.close()

    nc.compile()
    return nc


def _kq_perm():
    """new feature order (hgrp, half, hrow, d32) <- orig (4hg+hr)*64+half*32+d."""
    idx = np.empty(1024, np.int64)
    n = 0
    for hg in range(4):
        for hf in range(2):
            for hr in range(4):
                for dd in range(32):
                    idx[n] = (4 * hg + hr) * 64 + hf * 32 + dd
                    n += 1
    return idx


def prep_host_inputs(inputs):
    import ml_dtypes
    f8 = ml_dtypes.float8_e4m3
    bfl = ml_dtypes.bfloat16
    f32 = np.float32

    x = np.asarray(inputs["x"], f32)
    qkv_w = np.asarray(inputs["qkv_w"], f32)
    qkv_b = np.asarray(inputs["qkv_b"], f32)
    proj_w = np.asarray(inputs["proj_w"], f32)
    proj_b = np.asarray(inputs["proj_b"], f32)
    fc1_w = np.asarray(inputs["fc1_w"], f32)
    fc1_b = np.asarray(inputs["fc1_b"], f32)
    fc2_w = np.asarray(inputs["fc2_w"], f32)
    fc2_b = np.asarray(inputs["fc2_b"], f32)
    ln1_w = np.asarray(inputs["ln1_w"], f32)
    ln1_b = np.asarray(inputs["ln1_b"], f32)
    ln2_w = np.asarray(inputs["ln2_w"], f32)
    ln2_b = np.asarray(inputs["ln2_b"], f32)

    wqkv = ln1_w[:, None] * qkv_w          # [1024, 3072]
    bqkv = qkv_b + ln1_b @ qkv_w           # [3072]
    perm = _kq_perm()
    wq = wqkv[:, 0:1024][:, perm] * 32.0
    wk = wqkv[:, 1024:2048][:, perm] * 32.0
    wv = wqkv[:, 2048:3072] * 32.0
    bq_full = np.concatenate([bqkv[0:1024][perm], bqkv[1024:2048][perm]]) * 32.0
    vb = bqkv[2048:3072] * 32.0

    wqkv_all = np.concatenate([wq, wk, wv], axis=1)      # [1024, 3072]
    wqkv_dev = np.ascontiguousarray(
        wqkv_all.reshape(8, 128, 24, 128).transpose(1, 2, 0, 3)).astype(f8)
    bq_dev = np.ascontiguousarray(bq_full.reshape(16, 128).T).astype(f32)

    wproj_dev = np.ascontiguousarray(
        (proj_w * 32.0).reshape(8, 128, 1024).transpose(1, 0, 2)).astype(f8)

    w1s = ln2_w[:, None] * fc1_w * 32.0
    w1q = w1s.astype(f8)
    w1r = (w1s - w1q.astype(f32)).astype(f8)   # subnormal residual
    b1_dev = (fc1_b + ln2_b @ fc1_w).astype(f32)
    w1_dev = np.ascontiguousarray(
        w1q.astype(f32).reshape(8, 128, 32, 128).transpose(1, 2, 0, 3)).astype(f8)
    w1lo_dev = np.ascontiguousarray(
        w1r.astype(f32).reshape(8, 128, 32, 128).transpose(1, 2, 0, 3)).astype(f8)

    w2s = fc2_w * 64.0
    w2q = w2s.astype(f8)
    w2r = (w2s - w2q.astype(f32)).astype(f8)
    w2_dev = np.ascontiguousarray(
        w2q.astype(f32).reshape(32, 128, 1024).transpose(1, 0, 2)).astype(f8)
    w2lo_dev = np.ascontiguousarray(
        w2r.astype(f32).reshape(32, 128, 1024).transpose(1, 0, 2)).astype(f8)

    shared = {
        "wqkv": wqkv_dev, "bq": bq_dev, "vbias": vb.astype(bfl),
        "wproj": wproj_dev, "bproj": (proj_b * 1024.0).astype(bfl),
        "w1": w1_dev, "w1lo": w1lo_dev, "b1": b1_dev,
        "w2": w2_dev, "w2lo": w2lo_dev, "b2": (fc2_b * 64.0).astype(bfl),
    }
    in_maps = []
    for c in range(N_CORES):
        b, half = c // 2, c % 2
        own = x[b, half * 1024:(half + 1) * 1024]
        other = x[b, (1 - half) * 1024:(2 - half) * 1024]
        xc = np.concatenate([own, other], axis=0).astype(bfl)
        in_maps.append({"x": np.ascontiguousarray(xc), **shared})
    return in_maps


# ---------------------------------------------------------------------------
# Cached PJRT runner (jit once, reuse across kernel() calls)
# ---------------------------------------------------------------------------
_CACHE = {}


def _get_runner():
    if "runner" in _CACHE:
        return _CACHE["runner"]

    from jax.experimental.shard_map import shard_map
    from jax.sharding import Mesh, PartitionSpec
    from concourse.bass2jax import (
        _bass_exec_p, install_neuronx_cc_hook, partition_id_tensor,
    )

    nc = build_nc()
    install_neuronx_cc_hook()

    partition_name = nc.partition_id_tensor.name if nc.partition_id_tensor else None
    in_names, out_names, out_avals, zero_shapes = [], [], [], []
    for alloc in nc.m.functions[0].allocations:
        if not isinstance(alloc, mybir.MemoryLocationSet):
            continue
        name = alloc.memorylocations[0].name
        if alloc.kind == "ExternalInput":
            if name != partition_name:
                in_names.append(name)
        elif alloc.kind == "ExternalOutput":
            shape = tuple(alloc.tensor_shape)
            dtype = mybir.dt.np(alloc.dtype)
            out_names.append(name)
            out_avals.append(jax.core.ShapedArray(shape, dtype))
            zero_shapes.append((shape, dtype))
    n_params = len(in_names)
    n_outs = len(out_names)
    all_in = list(in_names) + list(out_names)
    if partition_name is not None:
        all_in.append(partition_name)
    donate = tuple(range(n_params, n_params + n_outs))

    def _body(*args):
        operands = list(args)
        if partition_name is not None:
            operands.append(partition_id_tensor())
        outs = _bass_exec_p.bind(
            *operands,
            out_avals=tuple(out_avals),
            in_names=tuple(all_in),
            out_names=tuple(out_names),
            lowering_input_output_aliases=(),
            sim_require_finite=True,
            sim_require_nnan=True,
            nc=nc,
        )
        return tuple(outs)

    devices = jax.devices()[:N_CORES]
    mesh = Mesh(np.asarray(devices), ("core",))
    sharded = jax.jit(
        shard_map(
            _body, mesh=mesh,
            in_specs=(PartitionSpec("core"),) * (n_params + n_outs),
            out_specs=(PartitionSpec("core"),) * n_outs,
            check_rep=False,
        ),
        donate_argnums=donate, keep_unused=True,
    )

    def run(in_maps):
        concat_in = [
            np.concatenate([np.asarray(m[name]) for m in in_maps], axis=0)
            for name in in_names
        ]
        concat_zeros = [
            np.zeros((N_CORES * s[0], *s[1:]), dt) for (s, dt) in zero_shapes
        ]
        out_arrs = sharded(*concat_in, *concat_zeros)
        per_core = []
        for c in range(N_CORES):
            per_core.append({
                name: np.asarray(out_arrs[i]).reshape(
                    N_CORES, *out_avals[i].shape)[c]
                for i, name in enumerate(out_names)
            })
        return per_core

    _CACHE["runner"] = run
    return run


def kernel(**inputs) -> np.ndarray:
    run = _get_runner()
    in_maps = prep_host_inputs(inputs)
    results = run(in_maps)
    out = np.zeros((4, 2048, 1024), np.float32)
    for c in range(N_CORES):
        b, half = c // 2, c % 2
        out[b, half * 1024:(half + 1) * 1024] = results[c]["y"]
    return out
